# revision 1
# baseline (speedup 1.0000x reference)
"""CoxNNet loss kernel for Trainium2 (8 NeuronCores, SPMD).

loss = -mean((theta - log(risk_sum)) * events) + 0.01 * ||W||_F
risk_sum[i] = sum_j exp(theta[j]) * (durations[j] >= durations[i])

Sharding: rows i are split 2048-per-core across 8 cores; every core holds the
full durations/theta vector for the j side.  Each core computes
  partial_c = -(1/n) * sum_{i in core} (theta_i - log risk_i) * events_i
            (+ 0.01*||W||_F on core 0 only, selected via an input flag)
and the host sums the 8 scalars.

Per-core layout: j on partitions ([128 x 128] tiles, chunk = column c holds
j = p*128 + c), i on the free axis (2048).  For each of the 128 j-chunks the
vector engine emits the risk-set mask (d_i <= d_j) as a bf16 [128, 2048] tile
(tensor_scalar is_le, per-partition scalar), and the tensor engine contracts
it with exp(theta_j) bf16 weights (M=1 matmuls, N=512), accumulating risk_sum
in four [1, 512] PSUM banks across all 128 chunks.

Hardware quirks worked around here (walrus codegen rejects >1 sync wait on
most 64B instruction structs, and the kernel-tail drain can only wait on a
limited number of semaphores):
  - inputs are host-packed so the kernel issues only 3 input DMAs;
  - each DMA'd tile is touched once by its consumer engine ("absorbers") so
    engine program order carries the dependency afterwards;
  - mask buffers are a self-managed 16-slice ring; a PE "heartbeat" matmul
    every 8 chunks plus a DVE copy that reads it ("fence") gives DVE a
    single-wait observation of PE progress for the WAR on ring reuse.
"""

import numpy as np

import concourse.bass as bass
import concourse.mybir as mybir
import concourse.tile as tile
from concourse.bass import ts
from concourse.bass_utils import run_bass_kernel_spmd
from concourse.tile_rust import add_dep_helper

F32 = mybir.dt.float32
BF16 = mybir.dt.bfloat16


class SplitDrainTileContext(tile.TileContext):
    """TileContext whose kernel-tail drain is split into one instruction per
    semaphore wait: this walrus build rejects any instruction carrying more
    than one sync-wait command ("Too many sync wait commands"), and the stock
    drain waits on every live semaphore at once.  Waits with values above 255
    are additionally split into stepped waits on the same semaphore."""

    def _drain_and_barrier(self, tick_clock, wait_clock):
        from concourse.vector_clock import ScopedClock

        drain_inst = self.nc.sync.drain()
        wait_clock.add_sem_waits(
            drain_inst.ins, ScopedClock({None: tick_clock.global_clock})
        )
        si = drain_inst.ins.sync_info
        if si is not None and si.on_wait:
            waits = []
            for w in si.on_wait:
                v = w.wait_value
                steps = list(range(255, v, 255)) + [v]
                for sv in steps:
                    waits.append(
                        mybir.SyncWait(
                            sync_type=w.sync_type,
                            id=w.id,
                            ant_name=w.ant_name,
                            wait_mode=w.wait_mode,
                            wait_value=sv,
                            wait_reg=w.wait_reg,
                        )
                    )
            drain_inst.ins.sync_info = mybir.SyncInfo(
                on_wait=waits[:1], on_update=list(si.on_update)
            )
            for w in waits[1:]:
                extra = self.nc.sync.drain()
                extra.ins.sync_info = mybir.SyncInfo(on_wait=[w], on_update=[])

        self.nc.all_engine_barrier()
        assert self.sems is not None
        popped = self.nc._tile_sem_poison_stack.pop()
        assert popped is self._sem_poison
        self.nc.clear_and_free_semaphores(list(self.sems.allocated().values()))
        self.nc.all_engine_barrier()

N = 16384
NCORES = 8
NI = N // NCORES          # rows per core
P = 128
JT = N // P               # j chunks per core
NSLICE = 512              # matmul free dim / one PSUM bank
NSL = NI // NSLICE        # matmul slices per chunk
L2_REG = 0.01
W_ROWS, W_COLS = 512, 256
WB = W_ROWS // P          # W row blocks
STG_COLS = JT + JT + WB * W_COLS   # staging: dur | theta | W
TAIL_COLS = 2 * NI + 16            # tail row: theta_i | events_i | flag | pad


def build(reps: int = 1) -> bass.Bass:
    """Build the per-core Bass kernel.  reps>1 repeats the O(n^2/8) main loop
    (re-starting PSUM accumulation each rep) for marginal-cost timing."""
    nc = bass.Bass()

    staging_in = nc.dram_tensor("staging_in", [P, STG_COLS], F32, kind="ExternalInput")
    tail_in = nc.dram_tensor("tail_in", [TAIL_COLS], F32, kind="ExternalInput")
    dur_i = nc.dram_tensor("dur_i", [NI], F32, kind="ExternalInput")
    out = nc.dram_tensor("out", [1, 1], F32, kind="ExternalOutput")

    with (
        SplitDrainTileContext(nc) as tc,
        tc.tile_pool(name="singles", bufs=1) as singles,
        tc.tile_pool(name="tail", bufs=1) as tail,
        tc.tile_pool(name="psum", bufs=1, space="PSUM") as psum,
    ):
        # ---- stage inputs (3 DMA ops -> 3 DMA queues/semaphores) ----
        staging = singles.tile([P, STG_COLS], F32, tag="staging")
        nc.sync.dma_start(out=staging, in_=staging_in.ap())
        dur_j = staging[:, 0:JT]
        theta_j = staging[:, JT : 2 * JT]
        w_sb = staging[:, 2 * JT : STG_COLS].rearrange("p (a c) -> p a c", a=WB)

        tailrow = singles.tile([1, TAIL_COLS], F32, tag="tailrow")
        nc.sync.dma_start(out=tailrow, in_=tail_in.ap().rearrange("(o n) -> o n", o=1))
        theta_i_sb = tailrow[:, 0:NI]
        events_sb = tailrow[:, NI : 2 * NI]
        flag_sb = tailrow[:, 2 * NI : 2 * NI + 1]

        # broadcast this core's row durations across all 128 partitions
        duri_b = singles.tile([P, NI], F32, tag="duri_b")
        dap = dur_i.ap()
        nc.sync.dma_start(
            out=duri_b,
            in_=bass.AP(tensor=dap.tensor, offset=dap.offset, ap=[[0, P]] + list(dap.ap)),
        )

        exp_bf = singles.tile([P, JT], BF16, tag="exp_bf")
        nc.scalar.activation(out=exp_bf, in_=theta_j, func=mybir.ActivationFunctionType.Exp)

        # ---- wait absorbers (see module docstring) ----
        scr_p = singles.tile([P, 4], F32, tag="scr_p")
        nc.vector.tensor_copy(scr_p[:, 0:1], staging[:, 0:1])
        nc.vector.tensor_copy(scr_p[:, 1:2], duri_b[:, 0:1])
        nc.vector.tensor_copy(scr_p[:1, 2:3], tailrow[:, 0:1])
        # PE heartbeat scratch banks (also serve as the PE-side absorber)
        scr_mm = [
            psum.tile([1, 1], F32, tag=f"scr_mm{k}", name=f"scr_mm{k}")
            for k in range(2)
        ]
        nc.tensor.matmul(scr_mm[1], exp_bf[:, 0:1], exp_bf[:, 0:1], start=True, stop=True)

        # ---- main O(n^2/8) loop ----
        RING = 16
        FK = RING // 2
        maskring = singles.tile([P, RING, NI], BF16, tag="maskring")
        fence_dst = singles.tile([1, 64], F32, tag="fence_dst")
        acc = [
            psum.tile([1, NSLICE], F32, tag=f"acc{s}", name=f"acc{s}")
            for s in range(NSL)
        ]
        fence_idx = 0
        last_fence = None
        for r in range(reps):
            for c in range(JT):
                slot = c % RING
                if c % FK == 0 and (c >= RING or r > 0):
                    # DVE observes "PE done reading the slices chunks
                    # c..c+FK-1 will overwrite" by reading the heartbeat
                    # PSUM written right after chunk c-FK-1's matmuls.
                    bank = ((c - RING) // FK) % 2
                    last_fence = nc.vector.tensor_copy(
                        fence_dst[:, (fence_idx % 64) : (fence_idx % 64) + 1],
                        scr_mm[bank],
                    ).ins
                    fence_idx += 1
                mask = maskring[:, slot, :]
                ts_ins = nc.vector.tensor_scalar(
                    out=mask,
                    in0=duri_b,
                    scalar1=dur_j[:, c : c + 1],
                    scalar2=None,
                    op0=mybir.AluOpType.is_le,
                ).ins
                if last_fence is not None:
                    add_dep_helper(ts_ins, last_fence, sync=False, reason="ring fence order")
                last_mm = None
                for s in range(NSL):
                    last_mm = nc.tensor.matmul(
                        acc[s],
                        exp_bf[:, c : c + 1],
                        mask[:, ts(s, NSLICE)],
                        start=(c == 0 and r == 0),
                        stop=(c == JT - 1 and r == reps - 1),
                    ).ins
                if c % FK == FK - 1:
                    # PE heartbeat: after chunk m = c, lets the fence at
                    # chunk m+FK+1 observe "PE consumed through chunk m"
                    bank = ((c - FK + 1) // FK) % 2
                    hb = nc.tensor.matmul(
                        scr_mm[bank],
                        exp_bf[:, 0:1],
                        exp_bf[:, 0:1],
                        start=True,
                        stop=True,
                    ).ins
                    add_dep_helper(hb, last_mm, sync=False, reason="heartbeat after chunk")

        # ---- tail: partial = sum((theta_i - ln risk) * events) ----
        lnr = tail.tile([1, NI], F32, tag="lnr")
        for s in range(NSL):
            nc.scalar.activation(
                out=lnr[:, ts(s, NSLICE)], in_=acc[s], func=mybir.ActivationFunctionType.Ln
            )
        tv = tail.tile([1, NI], F32, tag="tv")
        nc.vector.tensor_sub(tv, theta_i_sb, lnr)
        nc.vector.tensor_mul(tv, tv, events_sb)
        lsum = tail.tile([1, 1], F32, tag="lsum")
        nc.vector.tensor_reduce(
            lsum, tv, axis=mybir.AxisListType.X, op=mybir.AluOpType.add
        )

        # ---- l2 = flag * sqrt(sum(W^2)); flag = L2_REG on core 0 only ----
        wsq = tail.tile([P, WB, W_COLS], F32, tag="wsq")
        nc.vector.tensor_mul(wsq, w_sb, w_sb)
        wrow = tail.tile([P, 1], F32, tag="wrow")
        nc.vector.tensor_reduce(
            wrow, wsq, axis=mybir.AxisListType.XY, op=mybir.AluOpType.add
        )
        ones = tail.tile([P, 1], F32, tag="ones")
        nc.vector.memset(ones, 1.0)
        wsum_ps = psum.tile([1, 1], F32, tag="wsum", name="wsum")
        nc.tensor.matmul(wsum_ps, wrow, ones, start=True, stop=True)
        # sqrt via exp(0.5*ln(s)) to stay in the natural_log_exp table set
        lnw = tail.tile([1, 1], F32, tag="lnw")
        nc.scalar.activation(out=lnw, in_=wsum_ps, func=mybir.ActivationFunctionType.Ln)
        l2v = tail.tile([1, 1], F32, tag="l2v")
        nc.scalar.activation(
            out=l2v, in_=lnw, func=mybir.ActivationFunctionType.Exp, scale=0.5
        )
        l2f = tail.tile([1, 1], F32, tag="l2f")
        nc.vector.tensor_mul(l2f, l2v, flag_sb)

        # out = (-1/N) * lsum + l2f
        final = tail.tile([1, 1], F32, tag="final")
        nc.scalar.activation(
            out=final,
            in_=lsum,
            func=mybir.ActivationFunctionType.Identity,
            bias=l2f[:, :],
            scale=-1.0 / N,
        )
        nc.sync.dma_start(out=out.ap(), in_=final)

    return nc


_NC_CACHE: dict[int, bass.Bass] = {}


def _get_nc(reps: int = 1) -> bass.Bass:
    if reps not in _NC_CACHE:
        _NC_CACHE[reps] = build(reps)
    return _NC_CACHE[reps]


def make_in_maps(hazard_pred, durations, events, W):
    theta = np.ascontiguousarray(np.reshape(hazard_pred, (-1,)), dtype=np.float32)
    durations = np.ascontiguousarray(durations, dtype=np.float32)
    events = np.ascontiguousarray(events, dtype=np.float32)
    W = np.ascontiguousarray(W, dtype=np.float32)

    w_t = np.transpose(W.reshape(WB, P, W_COLS), (1, 0, 2)).reshape(P, WB * W_COLS)
    staging = np.concatenate(
        [durations.reshape(P, JT), theta.reshape(P, JT), w_t], axis=1
    ).astype(np.float32)
    staging = np.ascontiguousarray(staging)

    in_maps = []
    for c in range(NCORES):
        sl = slice(c * NI, (c + 1) * NI)
        tailrow = np.zeros([TAIL_COLS], dtype=np.float32)
        tailrow[0:NI] = theta[sl]
        tailrow[NI : 2 * NI] = events[sl]
        tailrow[2 * NI] = L2_REG if c == 0 else 0.0
        in_maps.append(
            {
                "staging_in": staging,
                "tail_in": tailrow,
                "dur_i": np.ascontiguousarray(durations[sl]),
            }
        )
    return in_maps


def run(in_maps, reps: int = 1):
    nc = _get_nc(reps)
    return run_bass_kernel_spmd(nc, in_maps, core_ids=list(range(NCORES)))


def kernel(hazard_pred, durations, events, W) -> np.ndarray:
    in_maps = make_in_maps(hazard_pred, durations, events, W)
    res = run(in_maps)
    total = np.zeros((), dtype=np.float64)
    for r in res.results:
        total += np.float64(r["out"].reshape(()))
    return np.asarray(total, dtype=np.float32)

